# revision 1
# baseline (speedup 1.0000x reference)
"""Trainium2 Bass kernel for DecoderWithAttention (location-sensitive attention decoder).

Strategy: data-parallel over batch (64 -> 8 cores x 8), weights replicated.
One SPMD program; all per-core data arrives via DRAM parameters.

Per-core layouts (SBUF):
  enc_m      [128(t%128), b, tc, d]      encoder outputs, for ctx matmuls (moving)
  enc_proj_s [128(a), b*1024+t]          W_enc @ enc, precomputed on-chip
  aw_n       [8(b), 2+1024+2]            normalized attention weights, zero halo
  aw_sh      [5(k), b*1024+t]            5 shifted copies of aw_n (conv unrolled)
  x_fullT    [128, 5*8]                  x^T blocks: [e;1;pad | ctx | h2], cols 8*xc+b
  cT_sb      [128, 2*8]                  LSTM cell state (fp32 master)
Gates are computed transposed: gT[128, gc, b], gate order [i, f, o, g].
h is stored doubled (h2 = (tanh(o/2)+1)*tanh(c) = 2h); every consumer weight
matrix is pre-scaled by 0.5 on the host.
Softmax uses a constant shift vbound >= max(energy) instead of a max-reduce;
length masking multiplies exp() by a 0/1 mask before the sum.
"""

import numpy as np

V, E, D_ENC, H, A, NF, KW = 96, 64, 256, 256, 128, 10, 5
B, T_ENC, T_TGT = 64, 1024, 256
TD = T_TGT - 1            # 255 decoder steps
NCORES = 8
BL = B // NCORES          # 8 batch elements per core

_prog_cache = {}


def _build_program(steps=TD, phases=99):
    import concourse.bass as bass
    import concourse.bacc as bacc
    import concourse.tile as tile
    from concourse import mybir
    from contextlib import ExitStack

    f32 = mybir.dt.float32
    f32r = mybir.dt.float32r
    AF = mybir.ActivationFunctionType
    OP = mybir.AluOpType


    nc = bacc.Bacc(None, target_bir_lowering=False)

    # ---- DRAM parameters (per-core) ----
    enc_d = nc.declare_dram_parameter("enc", [BL, T_ENC, D_ENC], f32r, isOutput=False)
    encT_d = nc.declare_dram_parameter("encT", [128, 2, BL * T_ENC], f32r,
                                       isOutput=False)
    wencT_d = nc.declare_dram_parameter("wencT", [128, 2, A], f32r, isOutput=False)
    mask_d = nc.declare_dram_parameter("mask01", [BL, T_ENC], f32, isOutput=False)
    oneh_d = nc.declare_dram_parameter("onehot", [V, steps * BL], f32, isOutput=False)
    wcomb_d = nc.declare_dram_parameter("wcomb", [128, 5, 8, 128], f32, isOutput=False)
    w1s_d = nc.declare_dram_parameter("w1s", [128, 4, 2, 128], f32, isOutput=False)
    w2t_d = nc.declare_dram_parameter("w2t", [128, 2, V], f32, isOutput=False)
    b1c_d = nc.declare_dram_parameter("b1c", [128, 2], f32, isOutput=False)
    b2r_d = nc.declare_dram_parameter("b2row", [1, V], f32, isOutput=False)
    wdecT_d = nc.declare_dram_parameter("wdecT", [128, 2, A], f32, isOutput=False)
    estat_d = nc.declare_dram_parameter("estat", [KW, A], f32r, isOutput=False)
    voh_d = nc.declare_dram_parameter("voh", [A, BL * BL], f32r, isOutput=False)
    emb_d = nc.declare_dram_parameter("emb", [V, E], f32, isOutput=False)
    eye_d = nc.declare_dram_parameter("eye128", [128, 128], f32, isOutput=False)
    eyer_d = nc.declare_dram_parameter("eye128r", [128, 128], f32r, isOutput=False)
    vb_d = nc.declare_dram_parameter("vbound", [BL, 1], f32, isOutput=False)
    out_d = nc.declare_dram_parameter("out", [BL, steps, V], f32, isOutput=True)

    with tile.TileContext(nc) as tc, ExitStack() as ctx:
        c1 = ctx.enter_context(tc.tile_pool(name="c1", bufs=1))

        # ---- persistent tiles ----
        enc_proj = c1.tile([128, BL * T_ENC], f32r)    # [a, b*1024+t]
        wcomb = c1.tile([128, 5, 8, 128], f32)
        w1s = c1.tile([128, 4, 2, 128], f32)
        w2t = c1.tile([128, 2, V], f32)
        b1c = c1.tile([128, 2], f32)
        b2r = c1.tile([1, V], f32)
        wdecT = c1.tile([128, 2, A], f32)
        estat = c1.tile([KW, A], f32r)
        voh = c1.tile([A, BL * BL], f32r)
        emb_s = c1.tile([V, E], f32)
        eye = c1.tile([128, 128], f32)
        eyer = c1.tile([128, 128], f32r)
        oneh = c1.tile([V, steps * BL], f32)
        mask01 = c1.tile([BL, T_ENC], f32)
        aw_n = c1.tile([BL, T_ENC + 4], f32r)          # halo cols 0:2 and 1026:1028
        aw_sh = c1.tile([KW, BL * T_ENC], f32r)
        awT8 = c1.tile([128, BL, 8, BL], f32r)         # [t%128, b, tc, j], col j==b holds aw
        om8 = c1.tile([128, BL, 8, BL], f32)           # static mask (j == b)
        x_fullT = c1.tile([128, 5 * BL], f32)
        cT = c1.tile([128, 2 * BL], f32)
        decT_s = c1.tile([128, BL], f32)
        scr_u = c1.tile([128, 2 * BL], f32)
        scr_w = c1.tile([128, 2 * BL], f32)
        scr_c2 = c1.tile([128, 2 * BL], f32)
        scr_tg = c1.tile([128, 2 * BL], f32)
        scr_tc = c1.tile([128, 2 * BL], f32)
        awraw = c1.tile([BL, T_ENC], f32)
        awm = c1.tile([BL, T_ENC], f32)
        sums = c1.tile([BL, 1], f32)
        rs = c1.tile([BL, 1], f32)
        ctx_sb = c1.tile([BL, D_ENC], f32)
        hidT = c1.tile([128, 2 * BL], f32)
        nvb = c1.tile([BL, 1], f32)
        ones8 = c1.tile([1, BL], f32)

        # ---- const loads ----
        for dst, src in [
            (wcomb, wcomb_d), (w1s, w1s_d), (w2t, w2t_d), (b1c, b1c_d),
            (b2r, b2r_d), (wdecT, wdecT_d), (estat, estat_d), (voh, voh_d),
            (emb_s, emb_d), (eye, eye_d), (eyer, eyer_d), (oneh, oneh_d),
            (mask01, mask_d),
        ]:
            nc.sync.dma_start(out=dst, in_=src[:])

        # -vbound, broadcast to a [1,1] sbuf tile for use as exp bias later.
        nc.sync.dma_start(out=nvb, in_=vb_d[:])

        # ---- preamble: enc_proj = W_enc @ enc^T, on-chip ----
        with tc.tile_pool(name="pre", bufs=1) as pre, \
             tc.tile_pool(name="prep", bufs=2, space="PSUM") as prep:
            wencT = pre.tile([128, 2, A], f32r)
            nc.sync.dma_start(out=wencT, in_=wencT_d[:])
            encT = pre.tile([128, 2, BL * T_ENC], f32r)  # [d%128, dc, b*1024+t]
            nc.sync.dma_start(out=encT, in_=encT_d[:])
            NBLK = BL * T_ENC // 512
            for blk in range(NBLK):
                ep_ps = prep.tile([128, 512], f32, tag="ep")
                for dc in range(2):
                    nc.tensor.matmul(
                        ep_ps[:, :], wencT[:, dc, :],
                        encT[:, dc, blk * 512:(blk + 1) * 512],
                        start=(dc == 0), stop=(dc == 1),
                    )
                nc.vector.tensor_copy(enc_proj[:, blk * 512:(blk + 1) * 512], ep_ps[:, :])

        # enc_m loaded after encT pool released (space reuse)
        c2 = ctx.enter_context(tc.tile_pool(name="c2", bufs=1))
        enc_m = c2.tile([128, BL, 8, D_ENC], f32r)
        for b in range(BL):
            nc.sync.dma_start(
                out=enc_m[:, b, :, :],
                in_=enc_d[b].rearrange("(tc p) d -> p tc d", p=128),
            )

        # ---- state init ----
        nc.vector.memset(aw_n.bitcast(f32), 0.0)
        nc.vector.memset(aw_n[:, 2:2 + T_ENC].bitcast(f32), 1.0 / T_ENC)
        nc.vector.memset(x_fullT, 0.0)
        nc.vector.memset(x_fullT[64:65, 0:BL], 1.0)
        nc.vector.memset(cT, 0.0)
        nc.vector.memset(ones8, 1.0)
        nc.vector.memset(om8, 0.0)
        for b in range(BL):
            nc.vector.memset(om8[:, b, :, b], 1.0)

        # ---- psum pools ----
        pein = ctx.enter_context(tc.tile_pool(name="pein", bufs=2, space="PSUM"))
        peng = ctx.enter_context(tc.tile_pool(name="peng", bufs=1, space="PSUM"))
        pmm = ctx.enter_context(tc.tile_pool(name="pmm", bufs=2, space="PSUM"))
        tpool = ctx.enter_context(tc.tile_pool(name="tanh", bufs=3))
        lgpool = ctx.enter_context(tc.tile_pool(name="lg", bufs=3))

        # helper APs over aw_n for the shift and transpose DMAs
        def aw_shift_src(k):
            # [8, 1024] view of aw_n starting at column k (shift k-2 with halo)
            base = aw_n[:, k:k + T_ENC]
            return base

        for t in range(steps):
            # (G') shift DMAs reading aw_{t-1}: aw_sh[k, b*1024+t] = aw_n[b, t+k]
            if phases >= 1:
                for k in range(KW):
                    nc.sync.dma_start(out=aw_sh[k:k + 1, :], in_=aw_shift_src(k))

            # (A) embedding via one-hot matmul -> eT into x_fullT block 0
            if phases < 2:
                lg = lgpool.tile([BL, V], f32, tag="lg")
                nc.vector.memset(lg, 0.0)
                nc.sync.dma_start(out=out_d[:, t, :], in_=lg)
                continue
            e_ps = pmm.tile([64, BL], f32, tag="mm")
            nc.tensor.matmul(e_ps[:, :], emb_s[:, :], oneh[:, t * BL:(t + 1) * BL],
                             start=True, stop=True)
            nc.vector.tensor_copy(x_fullT[0:64, 0:BL], e_ps[:, :])

            if phases < 3:
                lg = lgpool.tile([BL, V], f32, tag="lg")
                nc.vector.memset(lg, 0.0)
                nc.sync.dma_start(out=out_d[:, t, :], in_=lg)
                continue
            # (B) gates
            g_ps = pmm.tile([128, 8, BL], f32, tag="mm")
            for gc in range(8):
                for xc in range(5):
                    nc.tensor.matmul(
                        g_ps[:, gc, :], wcomb[:, xc, gc, :],
                        x_fullT[:, xc * BL:(xc + 1) * BL],
                        start=(xc == 0), stop=(xc == 4),
                    )

            if phases < 4:
                lg = lgpool.tile([BL, V], f32, tag="lg")
                nc.vector.tensor_copy(lg, g_ps[0:BL, 0, :][:, 0:V])
                nc.sync.dma_start(out=out_d[:, t, :], in_=lg)
                continue
            # (C) LSTM pointwise. gates order [i(0:2), f(2:4), o(4:6), g(6:8)]
            nc.scalar.activation(g_ps[:, 0:6, :], g_ps[:, 0:6, :], AF.Tanh, scale=0.5)
            nc.scalar.activation(scr_tg, g_ps[:, 6:8, :], AF.Tanh)
            nc.vector.scalar_tensor_tensor(scr_u, g_ps[:, 2:4, :], 1.0, cT,
                                           OP.add, OP.mult)
            nc.vector.scalar_tensor_tensor(scr_w, g_ps[:, 0:2, :], 1.0, scr_tg,
                                           OP.add, OP.mult)
            nc.vector.tensor_tensor(scr_c2, scr_u, scr_w, OP.add)
            nc.vector.tensor_scalar_mul(cT, scr_c2, 0.5)
            nc.scalar.activation(scr_tc, cT, AF.Tanh)
            nc.vector.scalar_tensor_tensor(x_fullT[:, 3 * BL:5 * BL], g_ps[:, 4:6, :],
                                           1.0, scr_tc, OP.add, OP.mult)

            if phases < 5:
                lg = lgpool.tile([BL, V], f32, tag="lg")
                nc.vector.memset(lg, 0.0)
                nc.sync.dma_start(out=out_d[:, t, :], in_=lg)
                continue
            # (D) decT = 0.5*W_dec @ h2
            d_ps = pmm.tile([128, BL], f32, tag="mm")
            for dc in range(2):
                nc.tensor.matmul(d_ps[:, :], wdecT[:, dc, :],
                                 x_fullT[:, (3 + dc) * BL:(4 + dc) * BL],
                                 start=(dc == 0), stop=(dc == 1))
            nc.vector.tensor_copy(decT_s, d_ps[:, :])

            if phases < 6:
                lg = lgpool.tile([BL, V], f32, tag="lg")
                nc.vector.memset(lg, 0.0)
                nc.sync.dma_start(out=out_d[:, t, :], in_=lg)
                continue
            # (E) energy pipeline + (F) softmax
            en_ps = peng.tile([BL, T_ENC], f32, tag="en")
            for b in range(BL):
                ei = pein.tile([128, T_ENC], f32, tag="ei")
                for ch in range(2):
                    cols = slice(b * T_ENC + ch * 512, b * T_ENC + (ch + 1) * 512)
                    nc.tensor.matmul(ei[:, ch * 512:(ch + 1) * 512],
                                     eyer[:, :], enc_proj[:, cols],
                                     start=True, stop=False)
                    nc.tensor.matmul(ei[:, ch * 512:(ch + 1) * 512],
                                     estat[:, :], aw_sh[:, cols],
                                     start=False, stop=True)
                th = tpool.tile([128, T_ENC], f32r, tag="th")
                nc.scalar.activation(th, ei[:, :], AF.Tanh,
                                     bias=decT_s[:, b:b + 1])
                for ch in range(2):
                    nc.tensor.matmul(en_ps[:, ch * 512:(ch + 1) * 512],
                                     voh[:, b * BL:(b + 1) * BL],
                                     th[:, ch * 512:(ch + 1) * 512],
                                     start=(b == 0), stop=(b == BL - 1))

            if phases < 7:
                lg = lgpool.tile([BL, V], f32, tag="lg")
                nc.vector.memset(lg, 0.0)
                nc.sync.dma_start(out=out_d[:, t, :], in_=lg)
                continue
            nc.scalar.activation(awraw, en_ps[:, :], AF.Exp, bias=nvb[:, 0:1])
            nc.vector.scalar_tensor_tensor(awm, awraw, 1.0, mask01,
                                           OP.mult, OP.mult, accum_out=sums)
            nc.vector.reciprocal(rs, sums)

            # (G) refresh aw_sh/awT happens at top of next iteration

            if phases < 8:
                lg = lgpool.tile([BL, V], f32, tag="lg")
                nc.vector.memset(lg, 0.0)
                nc.sync.dma_start(out=out_d[:, t, :], in_=lg)
                continue
            # (H) ctx[b, :] = sum_t aw[b, t] * enc[b, t, :]
            # Transpose aw via PE into [t%128, tc, b], mask to one-hot columns
            # (awT8[:, b, tc, j] = aw * (j==b)), then M=8 accumulating matmuls:
            # each writes ctx into row b and zeros elsewhere.
            aT_ps = pmm.tile([128, 8, BL], f32, tag="mm")
            for tcb in range(8):
                nc.tensor.transpose(
                    aT_ps[:, tcb, :],
                    awm[:, tcb * 128:(tcb + 1) * 128],
                    eye[0:BL, 0:BL])
            aT_bc = bass.AP(
                tensor=aT_ps[:, :, :].tensor, offset=aT_ps[:, :, :].offset,
                ap=[list(aT_ps[:, :, :].ap[0]), [0, BL], [BL, 8], [1, BL]])
            nc.vector.tensor_tensor(awT8, aT_bc, om8, OP.mult)
            cx_ps = pmm.tile([BL, D_ENC], f32, tag="mm")
            for b in range(BL):
                for tcb in range(8):
                    nc.tensor.matmul(
                        cx_ps[:, :], awT8[:, b, tcb, :],
                        enc_m[:, b, tcb, :],
                        start=(b == 0 and tcb == 0), stop=(b == BL - 1 and tcb == 7),
                    )
            nc.vector.tensor_scalar_mul(ctx_sb, cx_ps[:, :], rs[:, 0:1])

            # (I) ctxT into x_fullT blocks 1-2 via PE transpose
            for hc in range(2):
                ct_ps = pmm.tile([128, BL], f32, tag="mm")
                nc.tensor.transpose(ct_ps[:, :], ctx_sb[:, hc * 128:(hc + 1) * 128],
                                    eye[0:BL, 0:BL])
                nc.vector.tensor_copy(x_fullT[:, (1 + hc) * BL:(2 + hc) * BL],
                                      ct_ps[:, :])
            # normalized aw only feeds next step's shift DMAs (conv input),
            # so it runs late, off the softmax->ctx critical path
            nc.vector.tensor_scalar_mul(aw_n[:, 2:2 + T_ENC], awm, rs[:, 0:1])

            if phases < 9:
                lg = lgpool.tile([BL, V], f32, tag="lg")
                nc.vector.memset(lg, 0.0)
                nc.sync.dma_start(out=out_d[:, t, :], in_=lg)
                continue
            # (J) output MLP
            h_ps = pmm.tile([128, 2, BL], f32, tag="mm")
            for hc in range(2):
                for xc in range(4):
                    nc.tensor.matmul(h_ps[:, hc, :], w1s[:, xc, hc, :],
                                     x_fullT[:, (1 + xc) * BL:(2 + xc) * BL],
                                     start=(xc == 0), stop=(xc == 3))
            for hc in range(2):
                nc.scalar.activation(hidT[:, hc * BL:(hc + 1) * BL], h_ps[:, hc, :],
                                     AF.Tanh, bias=b1c[:, hc:hc + 1])
            l_ps = pmm.tile([BL, V], f32, tag="mm")
            for hc in range(2):
                nc.tensor.matmul(l_ps[:, :], hidT[:, hc * BL:(hc + 1) * BL],
                                 w2t[:, hc, :], start=(hc == 0), stop=False)
            nc.tensor.matmul(l_ps[:, :], ones8[:, :], b2r[:, :],
                             start=False, stop=True)
            lg = lgpool.tile([BL, V], f32, tag="lg")
            nc.vector.tensor_copy(lg, l_ps[:, :])
            nc.sync.dma_start(out=out_d[:, t, :], in_=lg)

    nc.compile()
    return nc


def _host_prep(inputs, core):
    """Build the per-core input map (all fp32 numpy)."""
    f = np.float32
    b0 = core * BL
    enc = np.ascontiguousarray(inputs["encoder_outputs"][b0:b0 + BL]).astype(f)
    lengths = np.asarray(inputs["encoder_lengths"][b0:b0 + BL])
    targets = np.asarray(inputs["targets"][b0:b0 + BL])
    emb = np.asarray(inputs["emb"]).astype(f)
    W_ih = np.asarray(inputs["W_ih"]).astype(f)
    W_hh = np.asarray(inputs["W_hh"]).astype(f)
    bias = (np.asarray(inputs["b_ih"]) + np.asarray(inputs["b_hh"])).astype(f)
    conv_w = np.asarray(inputs["conv_w"]).astype(f)
    W_enc = np.asarray(inputs["W_enc"]).astype(f)
    W_dec = np.asarray(inputs["W_dec"]).astype(f)
    W_loc = np.asarray(inputs["W_loc"]).astype(f)
    v_w = np.asarray(inputs["v_w"]).astype(f)
    out_w1 = np.asarray(inputs["out_w1"]).astype(f)
    out_b1 = np.asarray(inputs["out_b1"]).astype(f)
    out_w2 = np.asarray(inputs["out_w2"]).astype(f)
    out_b2 = np.asarray(inputs["out_b2"]).astype(f)

    # gate reorder [i, f, g, o] -> [i, f, o, g]
    perm = np.concatenate([np.arange(0, 512), np.arange(768, 1024),
                           np.arange(512, 768)])
    Wg = np.concatenate([W_ih, W_hh], axis=1)[perm]      # [1024, 576]
    bias2 = bias[perm]
    Wx = np.zeros((640, 4 * H), f)
    Wx[0:64] = Wg[:, 0:64].T
    Wx[64] = bias2
    Wx[128:384] = Wg[:, 64:320].T
    Wx[384:640] = 0.5 * Wg[:, 320:576].T
    wcomb = np.ascontiguousarray(
        Wx.reshape(5, 128, 1024).transpose(1, 0, 2).reshape(128, 5, 8, 128))

    W1x = np.zeros((512, H), f)
    W1x[0:256] = out_w1[:, H:H + D_ENC].T          # ctx part
    W1x[256:512] = 0.5 * out_w1[:, 0:H].T          # h part (h stored doubled)
    w1s = np.ascontiguousarray(
        W1x.reshape(4, 128, H).transpose(1, 0, 2).reshape(128, 4, 2, 128))

    w2t = np.ascontiguousarray(
        out_w2.T.reshape(2, 128, V).transpose(1, 0, 2))   # [128, 2, V]
    b1cc = np.ascontiguousarray(out_b1.reshape(2, 128).T)  # [128, 2]
    wdecT = np.ascontiguousarray(
        (0.5 * W_dec.T).reshape(2, 128, A).transpose(1, 0, 2))  # [128, 2, A]
    wencT = np.ascontiguousarray(
        W_enc.T.reshape(2, 128, A).transpose(1, 0, 2))  # [128, 2, A]

    M = W_loc @ conv_w[:, 0, :]                    # [A, KW]
    estat = np.ascontiguousarray(M.T)              # [KW, A]

    v = v_w[0]                                     # [A]
    voh = np.zeros((A, BL * BL), f)
    for b in range(BL):
        voh[:, b * BL + b] = v
    vbound = np.float32(np.abs(v).sum() + 1.0)

    mask01 = (np.arange(T_ENC)[None, :] < lengths[:, None]).astype(f)

    tgt = targets[:, :TD]                          # teacher forcing inputs
    onehot = np.zeros((V, TD * BL), f)
    cols = np.arange(TD * BL)
    onehot[tgt.T.reshape(-1), cols] = 1.0          # col t*BL+b -> targets[b, t]

    encT = np.ascontiguousarray(
        enc.reshape(BL * T_ENC, D_ENC).T.reshape(2, 128, BL * T_ENC)
        .transpose(1, 0, 2))

    return {
        "enc": enc,
        "encT": encT,
        "wencT": wencT,
        "mask01": mask01,
        "onehot": onehot,
        "wcomb": wcomb,
        "w1s": w1s,
        "w2t": w2t,
        "b1c": b1cc,
        "b2row": out_b2[None, :].astype(f),
        "wdecT": wdecT,
        "estat": estat,
        "voh": voh,
        "emb": emb,
        "eye128": np.eye(128, dtype=f),
        "eye128r": np.eye(128, dtype=f),
        "vbound": np.full((BL, 1), -vbound, f),
    }


def kernel(**inputs) -> np.ndarray:
    from concourse.bass_utils import run_bass_kernel_spmd

    if "prog" not in _prog_cache:
        _prog_cache["prog"] = _build_program()
    nc = _prog_cache["prog"]

    in_maps = [_host_prep(inputs, c) for c in range(NCORES)]
    res = run_bass_kernel_spmd(nc, in_maps, list(range(NCORES)))
    outs = [res.results[c]["out"] for c in range(NCORES)]
    return np.concatenate(outs, axis=0).astype(np.float32)


if __name__ == "__main__":
    import reference
    inputs = {k: np.asarray(v) for k, v in reference.setup_inputs().items()}
    got = kernel(**inputs)
    exp = np.asarray(reference.reference(**reference.setup_inputs()))
    err = np.abs(got - exp).max() / (np.abs(exp).max() + 1e-30)
    print("Relative error:", err)



# revision 7
# speedup vs baseline: 1.6229x; 1.6229x over previous
"""Trainium2 Bass kernel for DecoderWithAttention (location-sensitive attention decoder).

Strategy: data-parallel over batch (64 -> 8 cores x 8), weights replicated.
One SPMD program; per-core data arrives via 3 DRAM input parameters:
  enc16 [8, 1024, 256] bf16, pk16 [128, PK] bf16 (weights/constants packed),
  pk32 [128, 1034] f32 (mask, exp bias, f32 identity). Output bf16.

Compute layout (per core, bf16 streams with f32 PSUM accumulation):
  x_fullT [128, 5*8]   x^T blocks: [e;1;pad | ctx0 | ctx1 | h2_0 | h2_1], col 8*xc+b
  gates   [8, 1024]    batch-major, 10 matmuls (x-chunks stationary, W moving)
  energy  [128(a), 1024(t)] psum per b: eye@enc_proj + stat13@aw13, where stat13
                       rows 0:8 = dec(b,:) (one-hot contraction) and 8:13 = conv
  ctx     aw transposed via PE, masked to one-hot cols, 64 accumulating matmuls
h is stored doubled (h2 = (tanh(o/2)+1)*tanh(c) = 2h); consumer weights pre-scaled 0.5.
Softmax uses a constant shift vbound >= max(energy); masking multiplies exp().
"""

import numpy as np

V, E, D_ENC, H, A, NF, KW = 96, 64, 256, 256, 128, 10, 5
B, T_ENC, T_TGT = 64, 1024, 256
TD = T_TGT - 1            # 255 decoder steps
NCORES = 8
BL = B // NCORES          # 8 batch elements per core

# pk16 column layout
C_EYE = 0                 # [128, 128] identity (bf16)
C_WENC = 128              # [128, 2*128] W_enc^T chunks
C_WDEC = 384              # [128, 2*128] 0.5*W_dec^T chunks
C_W2 = 640                # [128, 2*96]  out_w2^T chunks
C_VOH = 832               # [128, 64]    v one-hot cols
C_ESTAT = 896             # [5, 128]     (W_loc @ conv)^T
C_ONES = 1024             # [1, 8] ones row
C_B2 = 1032               # [1, 96] out_b2
C_B1 = 1128               # [1, 256] out_b1
C_W1 = 1384               # [128, 4*256] out_w1^T chunks (ctx0 ctx1 h0 h1)
C_TOK = 2408              # [64, steps*8] token embeddings^T

# pk32 column layout: 0:1024 mask (rows 0:8), 1024 nvb (rows 0:8), 1026:1034 eye8 f32


def _pk_cols(steps):
    c_wg = C_TOK + steps * BL
    return c_wg, c_wg + 5 * 1024


_prog_cache = {}


def _build_program(steps=TD):
    import concourse.bass as bass
    import concourse.bacc as bacc
    import concourse.tile as tile
    from concourse import mybir
    from contextlib import ExitStack

    f32 = mybir.dt.float32
    bf16 = mybir.dt.bfloat16
    AF = mybir.ActivationFunctionType
    OP = mybir.AluOpType

    C_WG, PK_COLS = _pk_cols(steps)

    nc = bacc.Bacc(None, target_bir_lowering=False)

    enc_d = nc.declare_dram_parameter("enc16", [BL, T_ENC, D_ENC], bf16,
                                      isOutput=False)
    pk_d = nc.declare_dram_parameter("pk16", [128, PK_COLS], bf16, isOutput=False)
    p32_d = nc.declare_dram_parameter("pk32", [128, 1034], f32, isOutput=False)
    out_d = nc.declare_dram_parameter("out", [BL, steps, V], bf16, isOutput=True)

    with tile.TileContext(nc) as tc, ExitStack() as ctx:
        c1 = ctx.enter_context(tc.tile_pool(name="c1", bufs=1))

        # ---- persistent tiles ----
        pk = c1.tile([128, PK_COLS], bf16)
        p32 = c1.tile([128, 1034], f32)
        enc_m = c1.tile([128, BL, 8, D_ENC], bf16)     # [t%128, b, tc, d]
        enc_proj = c1.tile([128, BL * T_ENC], bf16)    # [a, b*1024+t]
        aw13 = c1.tile([13, BL * T_ENC], bf16)         # rows 0:8 onehot(b), 8:13 shifts
        stat13 = c1.tile([13, A], bf16)                # rows 0:8 dec, 8:13 estat
        awe32 = c1.tile([BL, T_ENC], f32)              # exp output
        awm32 = c1.tile([BL, T_ENC], f32)              # exp * mask (unnormalized)
        aw_n = c1.tile([BL, T_ENC + 4], bf16)          # normalized aw, zero halo
        awT8 = c1.tile([128, BL, 8, BL], bf16)         # [t%128, b, tc, j]
        om8 = c1.tile([128, BL, 8, BL], f32)           # static mask (j == b)
        x_fullT = c1.tile([128, 5 * BL], bf16)
        cT = c1.tile([BL, H], f32)                     # LSTM cell state
        tg = c1.tile([BL, H], f32)
        tc_s = c1.tile([BL, H], f32)
        scr_u = c1.tile([BL, H], f32)
        scr_w = c1.tile([BL, H], f32)
        h2 = c1.tile([BL, H], f32)
        hid_sb = c1.tile([BL, H], f32)
        hidT = c1.tile([128, 2 * BL], bf16)
        ctx_sb = c1.tile([BL, D_ENC], f32)
        sums = c1.tile([BL, 1], f32)
        rs = c1.tile([BL, 1], f32)

        # pk views
        eye16 = pk[:, C_EYE:C_EYE + 128]

        def wenc(dc):
            return pk[:, C_WENC + dc * 128:C_WENC + (dc + 1) * 128]

        def wdec(dc):
            return pk[:, C_WDEC + dc * 128:C_WDEC + (dc + 1) * 128]

        def w2(hc):
            return pk[:, C_W2 + hc * V:C_W2 + (hc + 1) * V]

        def voh(b):
            return pk[:, C_VOH + b * BL:C_VOH + (b + 1) * BL]

        ones_r = pk[0:1, C_ONES:C_ONES + BL]
        b2_r = pk[0:1, C_B2:C_B2 + V]
        b1_r = pk[0:1, C_B1:C_B1 + H]

        def w1T(xc):
            return pk[:, C_W1 + xc * H:C_W1 + (xc + 1) * H]

        def tok(t):
            return pk[0:64, C_TOK + t * BL:C_TOK + (t + 1) * BL]

        def wg(xc, half):
            o = C_WG + xc * 1024 + half * 512
            return pk[:, o:o + 512]

        mask32 = p32[0:BL, 0:T_ENC]
        nvb = p32[0:BL, 1024:1025]
        eye8 = p32[0:BL, 1026:1026 + BL]

        # ---- const loads ----
        nc.sync.dma_start(out=pk, in_=pk_d[:])
        nc.sync.dma_start(out=p32, in_=p32_d[:])
        for b in range(BL):
            nc.sync.dma_start(
                out=enc_m[:, b, :, :],
                in_=enc_d[b].rearrange("(tc p) d -> p tc d", p=128),
            )
        # estat into stat13 rows 8:13 (partition-shifting SBUF->SBUF DMA)
        nc.sync.dma_start(out=stat13[8:13, :], in_=pk[0:5, C_ESTAT:C_ESTAT + 128])

        # ---- state init ----
        nc.vector.memset(aw_n, 0.0)
        nc.vector.memset(aw_n[:, 2:2 + T_ENC], 1.0 / T_ENC)
        nc.vector.memset(x_fullT, 0.0)
        nc.vector.memset(x_fullT[64:65, 0:BL], 1.0)
        nc.vector.memset(cT, 0.0)
        nc.vector.memset(om8, 0.0)
        nc.vector.memset(aw13, 0.0)
        onesrow = c1.tile([1, T_ENC], bf16)
        nc.vector.memset(onesrow, 1.0)
        for b in range(BL):
            nc.vector.memset(om8[:, b, :, b], 1.0)
            # one-hot conv rows; DMA because DVE can't target partition b
            nc.sync.dma_start(out=aw13[b:b + 1, b * T_ENC:(b + 1) * T_ENC],
                              in_=onesrow)

        # ---- preamble: encT via DMA transposes, then enc_proj = W_enc @ enc^T ----
        with tc.tile_pool(name="pre", bufs=1) as pre, \
             tc.tile_pool(name="prep", bufs=2, space="PSUM") as prep:
            encT = pre.tile([128, 2, BL * T_ENC], bf16)   # [d%128, dc, b*1024+t]
            for b in range(BL):
                for tcb in range(8):
                    for dc in range(2):
                        nc.sync.dma_start(
                            out=encT[:, dc, b * T_ENC + tcb * 128:
                                     b * T_ENC + (tcb + 1) * 128],
                            in_=enc_m[:, b, tcb, dc * 128:(dc + 1) * 128],
                            transpose=True,
                        )
            for blk in range(BL * T_ENC // 512):
                ep_ps = prep.tile([128, 512], f32, tag="ep")
                for dc in range(2):
                    nc.tensor.matmul(
                        ep_ps[:, :], wenc(dc),
                        encT[:, dc, blk * 512:(blk + 1) * 512],
                        start=(dc == 0), stop=(dc == 1),
                    )
                nc.vector.tensor_copy(enc_proj[:, blk * 512:(blk + 1) * 512],
                                      ep_ps[:, :])

        # initial shift rows of aw13 from the uniform aw_n
        for k in range(KW):
            nc.sync.dma_start(out=aw13[8 + k:9 + k, :], in_=aw_n[:, k:k + T_ENC])

        # ---- psum pools ----
        pein = ctx.enter_context(tc.tile_pool(name="pein", bufs=2, space="PSUM"))
        pmed = ctx.enter_context(tc.tile_pool(name="pmed", bufs=2, space="PSUM"))
        pmm = ctx.enter_context(tc.tile_pool(name="pmm", bufs=2, space="PSUM"))
        tpool = ctx.enter_context(tc.tile_pool(name="tanh", bufs=3))
        lgpool = ctx.enter_context(tc.tile_pool(name="lg", bufs=3))

        def seed_chunk(b):
            """Open energy psum chunk b and seed it with enc_proj via identity."""
            ei = pein.tile([128, T_ENC], f32, tag="ei")
            for ch in range(2):
                nc.tensor.matmul(ei[:, ch * 512:(ch + 1) * 512], eye16,
                                 enc_proj[:, b * T_ENC + ch * 512:
                                          b * T_ENC + (ch + 1) * 512],
                                 start=True, stop=False)
            return ei

        for t in range(steps):
            # (A) token embedding columns for this step
            nc.vector.tensor_copy(x_fullT[0:64, 0:BL], tok(t))

            # (B) gates batch-major: lo=[i,f], hi=[o,g]
            gp_lo = pmed.tile([BL, 512], f32, tag="md")
            gp_hi = pmed.tile([BL, 512], f32, tag="md")
            for half, gp in ((0, gp_lo), (1, gp_hi)):
                for xc in range(5):
                    nc.tensor.matmul(
                        gp[:, :], x_fullT[:, xc * BL:(xc + 1) * BL],
                        wg(xc, half), start=(xc == 0), stop=(xc == 4),
                    )

            # first two energy chunks only need enc_proj: overlap the pointwise
            ei0 = seed_chunk(0)
            ei1 = seed_chunk(1)

            # (C) LSTM pointwise in [8, 256] space; i' = tanh(i/2) etc.
            nc.scalar.activation(gp_lo[:, :], gp_lo[:, :], AF.Tanh, scale=0.5)
            nc.scalar.activation(tg, gp_hi[:, H:2 * H], AF.Tanh)
            nc.scalar.activation(gp_hi[:, 0:H], gp_hi[:, 0:H], AF.Tanh, scale=0.5)
            nc.vector.scalar_tensor_tensor(scr_u, gp_lo[:, H:2 * H], 1.0, cT,
                                           OP.add, OP.mult)
            nc.vector.scalar_tensor_tensor(scr_w, gp_lo[:, 0:H], 1.0, tg,
                                           OP.add, OP.mult)
            nc.vector.tensor_tensor(scr_u, scr_u, scr_w, OP.add)
            nc.vector.tensor_scalar_mul(cT, scr_u, 0.5)
            nc.scalar.activation(tc_s, cT, AF.Tanh)
            nc.vector.scalar_tensor_tensor(h2, gp_hi[:, 0:H], 1.0, tc_s,
                                           OP.add, OP.mult)

            # (D) h2 -> x_fullT blocks 3,4 via PE transpose (f32 in, bf16 out)
            for hc in range(2):
                hT_ps = pmm.tile([128, BL], f32, tag="mm")
                nc.tensor.transpose(hT_ps[:, :], h2[:, hc * 128:(hc + 1) * 128],
                                    eye8)
                nc.vector.tensor_copy(x_fullT[:, (3 + hc) * BL:(4 + hc) * BL],
                                      hT_ps[:, :])
            # dec[b, a] computed directly transposed: lhsT = h2T chunks
            dT_ps = pmm.tile([BL, A], f32, tag="mm")
            for dc in range(2):
                nc.tensor.matmul(dT_ps[:, :], x_fullT[:, (3 + dc) * BL:(4 + dc) * BL],
                                 wdec(dc), start=(dc == 0), stop=(dc == 1))
            nc.vector.tensor_copy(stat13[0:BL, :], dT_ps[:, :])

            # (E) energy pipeline: per b chunk: psum = enc_proj + stat13 @ aw13
            en_lo = pmed.tile([BL, 512], f32, tag="md")
            en_hi = pmed.tile([BL, 512], f32, tag="md")
            for b in range(BL):
                ei = (ei0 if b == 0 else ei1 if b == 1 else seed_chunk(b))
                for ch in range(2):
                    cols = slice(b * T_ENC + ch * 512, b * T_ENC + (ch + 1) * 512)
                    nc.tensor.matmul(ei[:, ch * 512:(ch + 1) * 512],
                                     stat13[:, :], aw13[:, cols],
                                     start=False, stop=True)
                th = tpool.tile([128, T_ENC], bf16, tag="th")
                nc.scalar.activation(th, ei[:, :], AF.Tanh)
                for ch, en in ((0, en_lo), (1, en_hi)):
                    nc.tensor.matmul(en[:, :], voh(b),
                                     th[:, ch * 512:(ch + 1) * 512],
                                     start=(b == 0), stop=(b == BL - 1))

            # (F) softmax (constant-shift exp, masked, unnormalized)
            nc.scalar.activation(awe32[:, 0:512], en_lo[:, :], AF.Exp,
                                 bias=nvb)
            nc.scalar.activation(awe32[:, 512:1024], en_hi[:, :], AF.Exp,
                                 bias=nvb)
            nc.vector.scalar_tensor_tensor(awm32, awe32, 1.0, mask32,
                                           OP.mult, OP.mult, accum_out=sums)
            nc.vector.reciprocal(rs, sums)

            # (G) aw transpose + one-hot masking for ctx matmuls
            aT_ps = pmm.tile([128, 8, BL], f32, tag="mm")
            for tcb in range(8):
                nc.tensor.transpose(aT_ps[:, tcb, :],
                                    awm32[:, tcb * 128:(tcb + 1) * 128], eye8)
            aT_bc = bass.AP(
                tensor=aT_ps[:, :, :].tensor, offset=aT_ps[:, :, :].offset,
                ap=[list(aT_ps[:, :, :].ap[0]), [0, BL], [BL, 8], [1, BL]])
            nc.vector.tensor_tensor(awT8, aT_bc, om8, OP.mult)

            # normalized aw feeds next step's conv rows (off critical path)
            nc.vector.tensor_scalar_mul(aw_n[:, 2:2 + T_ENC], awm32, rs[:, 0:1])
            for k in range(KW):
                nc.sync.dma_start(out=aw13[8 + k:9 + k, :],
                                  in_=aw_n[:, k:k + T_ENC])

            # (H) ctx = aw @ enc (unnormalized; scale by rs after)
            cx_ps = pmed.tile([BL, D_ENC], f32, tag="md")
            for b in range(BL):
                for tcb in range(8):
                    nc.tensor.matmul(
                        cx_ps[:, :], awT8[:, b, tcb, :], enc_m[:, b, tcb, :],
                        start=(b == 0 and tcb == 0), stop=(b == BL - 1 and tcb == 7),
                    )
            nc.vector.tensor_scalar_mul(ctx_sb, cx_ps[:, :], rs[:, 0:1])

            # (I) ctx -> x_fullT blocks 1,2
            for hc in range(2):
                cT_ps = pmm.tile([128, BL], f32, tag="mm")
                nc.tensor.transpose(cT_ps[:, :], ctx_sb[:, hc * 128:(hc + 1) * 128],
                                    eye8)
                nc.vector.tensor_copy(x_fullT[:, (1 + hc) * BL:(2 + hc) * BL],
                                      cT_ps[:, :])

            # (J) output MLP: hid = tanh(W1 @ [ctx; h] + b1), batch-major
            hid_ps = pmed.tile([BL, H], f32, tag="md")
            for xc in range(4):
                nc.tensor.matmul(hid_ps[:, :], x_fullT[:, (1 + xc) * BL:(2 + xc) * BL],
                                 w1T(xc), start=(xc == 0), stop=False)
            nc.tensor.matmul(hid_ps[:, :], ones_r, b1_r,
                             start=False, stop=True)
            nc.scalar.activation(hid_sb, hid_ps[:, :], AF.Tanh)
            for hc in range(2):
                hT_ps = pmm.tile([128, BL], f32, tag="mm")
                nc.tensor.transpose(hT_ps[:, :], hid_sb[:, hc * 128:(hc + 1) * 128],
                                    eye8)
                nc.vector.tensor_copy(hidT[:, hc * BL:(hc + 1) * BL], hT_ps[:, :])
            l_ps = pmed.tile([BL, V], f32, tag="md")
            for hc in range(2):
                nc.tensor.matmul(l_ps[:, :], hidT[:, hc * BL:(hc + 1) * BL],
                                 w2(hc), start=(hc == 0), stop=False)
            nc.tensor.matmul(l_ps[:, :], ones_r, b2_r, start=False, stop=True)
            lg = lgpool.tile([BL, V], bf16, tag="lg")
            nc.vector.tensor_copy(lg, l_ps[:, :])
            nc.sync.dma_start(out=out_d[:, t, :], in_=lg)

    nc.compile()
    return nc


def _host_prep(inputs, core, steps=TD):
    """Build the per-core input map."""
    import ml_dtypes

    f = np.float32
    bf = ml_dtypes.bfloat16
    C_WG, PK_COLS = _pk_cols(steps)
    b0 = core * BL

    enc = np.asarray(inputs["encoder_outputs"][b0:b0 + BL], f)
    lengths = np.asarray(inputs["encoder_lengths"][b0:b0 + BL])
    targets = np.asarray(inputs["targets"][b0:b0 + BL])
    emb = np.asarray(inputs["emb"], f)
    W_ih = np.asarray(inputs["W_ih"], f)
    W_hh = np.asarray(inputs["W_hh"], f)
    bias = (np.asarray(inputs["b_ih"]) + np.asarray(inputs["b_hh"])).astype(f)
    conv_w = np.asarray(inputs["conv_w"], f)
    W_enc = np.asarray(inputs["W_enc"], f)
    W_dec = np.asarray(inputs["W_dec"], f)
    W_loc = np.asarray(inputs["W_loc"], f)
    v = np.asarray(inputs["v_w"], f)[0]
    out_w1 = np.asarray(inputs["out_w1"], f)
    out_b1 = np.asarray(inputs["out_b1"], f)
    out_w2 = np.asarray(inputs["out_w2"], f)
    out_b2 = np.asarray(inputs["out_b2"], f)

    pk = np.zeros((128, PK_COLS), bf)
    pk[:, C_EYE:C_EYE + 128] = np.eye(128, dtype=f)
    pk[:, C_WENC:C_WENC + 256] = W_enc.T.reshape(2, 128, A).transpose(1, 0, 2) \
        .reshape(128, 256)
    pk[:, C_WDEC:C_WDEC + 256] = (0.5 * W_dec.T).reshape(2, 128, A) \
        .transpose(1, 0, 2).reshape(128, 256)
    pk[:, C_W2:C_W2 + 2 * V] = out_w2.T.reshape(2, 128, V).transpose(1, 0, 2) \
        .reshape(128, 2 * V)
    vohm = np.zeros((A, BL * BL), f)
    for b in range(BL):
        vohm[:, b * BL + b] = v
    pk[:, C_VOH:C_VOH + 64] = vohm
    M = W_loc @ conv_w[:, 0, :]                     # [A, KW]
    pk[0:5, C_ESTAT:C_ESTAT + 128] = M.T
    pk[0, C_ONES:C_ONES + BL] = 1.0
    pk[0, C_B2:C_B2 + V] = out_b2
    pk[0, C_B1:C_B1 + H] = out_b1
    # w1 chunks: x blocks 1,2 = ctx, 3,4 = h2 (0.5-scaled)
    w1x = np.zeros((4, 128, H), f)
    w1x[0] = out_w1[:, H + 0:H + 128].T
    w1x[1] = out_w1[:, H + 128:H + 256].T
    w1x[2] = 0.5 * out_w1[:, 0:128].T
    w1x[3] = 0.5 * out_w1[:, 128:256].T
    pk[:, C_W1:C_W1 + 4 * H] = w1x.transpose(1, 0, 2).reshape(128, 4 * H)
    # token embeddings^T: col t*8+b
    tokT = emb[targets[:, :steps]]                  # [BL, steps, E]
    pk[0:64, C_TOK:C_TOK + steps * BL] = tokT.transpose(2, 1, 0).reshape(E, -1)
    # gate weights, gate order [i, f, o, g]; moving layout [128, 5, 1024]
    perm = np.concatenate([np.arange(0, 512), np.arange(768, 1024),
                           np.arange(512, 768)])
    Wg = np.concatenate([W_ih, W_hh], axis=1)[perm]      # [1024, 576]
    bias2 = bias[perm]
    Wg2 = np.zeros((640, 4 * H), f)
    Wg2[0:64] = Wg[:, 0:64].T
    Wg2[64] = bias2
    Wg2[128:384] = Wg[:, 64:320].T
    Wg2[384:640] = 0.5 * Wg[:, 320:576].T
    pk[:, C_WG:C_WG + 5 * 1024] = Wg2.reshape(5, 128, 1024).transpose(1, 0, 2) \
        .reshape(128, 5 * 1024)

    p32 = np.zeros((128, 1034), f)
    p32[0:BL, 0:T_ENC] = np.arange(T_ENC)[None, :] < np.asarray(lengths)[:, None]
    vbound = np.float32(np.abs(v).sum() + 1.0)
    p32[0:BL, 1024] = -vbound
    p32[0:BL, 1026:1026 + BL] = np.eye(BL, dtype=f)

    return {
        "enc16": enc.astype(bf),
        "pk16": pk,
        "pk32": p32,
    }


def kernel(**inputs) -> np.ndarray:
    from concourse.bass_utils import run_bass_kernel_spmd

    if "prog" not in _prog_cache:
        _prog_cache["prog"] = _build_program()
    nc = _prog_cache["prog"]

    in_maps = [_host_prep(inputs, c) for c in range(NCORES)]
    res = run_bass_kernel_spmd(nc, in_maps, list(range(NCORES)))
    outs = [res.results[c]["out"].astype(np.float32) for c in range(NCORES)]
    return np.concatenate(outs, axis=0)


if __name__ == "__main__":
    import reference
    inputs = {k: np.asarray(v) for k, v in reference.setup_inputs().items()}
    got = kernel(**inputs)
    exp = np.asarray(reference.reference(**reference.setup_inputs()))
    err = np.abs(got - exp).max() / (np.abs(exp).max() + 1e-30)
    print("Relative error:", err)
